# revision 11
# baseline (speedup 1.0000x reference)
"""AdaptiveMixGNNLayer Trainium2 kernel (8 NeuronCores, SPMD, no collectives).

Strategy: 1D node partition — each core owns a contiguous range of destination
rows (rpc = N/8). Host (inside kernel()) partitions+sorts the COO edges of both
operators by (core, 128-row destination block, x-half), pads each segment to
whole 128-edge tiles, and ships per-core int16 gather indices + bf16 edge
values. x is replicated in each core's HBM as two bf16 halves (dma_gather
indices are int16, so the gather table must stay under 32768 rows).

Per destination block on device:
  - dma_gather pulls x[col] for all the block's edges from the low half, then
    the high half (edge -> partition i%128, tile -> free slot i//128)
  - DVE builds one-hot P'[e, (j, t)] = val * (rblk == j) from a repeated iota
    (j-major so every operand's last AP dim is packed) via broadcast APs
  - TensorE accumulates Z[r, :] += P'_t^T @ Xg_t into PSUM (lp / hp separate)
  - epilogue: alpha-mix (per-row scale on ACT), TensorE transpose, @ W^T, +b,
    ReLU, DMA out.
alpha = sigmoid(x @ alpha_w^T + alpha_b) is computed on-device in f32 from the
core's own row slice.
"""
import numpy as np

P = 128  # partitions / tile edge


# ---------------------------------------------------------------- host prep
def _prep_op(rows, cols, vals, n_cores, rpc, nblk, half):
    """Sort one operator's edges by (core, dest block, col half).

    Returns (tlo, thi) per-block tile counts (max over cores, SPMD-shared) and
    per-(core, block, half) segment slices of the sorted arrays.
    """
    rows = np.asarray(rows)
    cols = np.asarray(cols)
    vals = np.asarray(vals)
    core = rows // rpc
    rloc = rows - core * rpc
    blk = rloc // P
    rblk = (rloc - blk * P).astype(np.float32)
    hi = (cols >= half).astype(np.int64)
    key = (core * nblk + blk) * 2 + hi
    order = np.argsort(key, kind="stable")
    cnt = np.bincount(key, minlength=n_cores * nblk * 2).reshape(
        n_cores, nblk, 2)
    tlo = -(-cnt[:, :, 0].max(axis=0) // P)
    thi = -(-cnt[:, :, 1].max(axis=0) // P)
    # every block needs >= 1 tile for this operator so its PSUM group exists
    empty = (tlo + thi) == 0
    tlo[empty] = 1
    gstart = np.concatenate([[0], np.cumsum(cnt.reshape(-1))])
    cs = cols[order].astype(np.int32)
    cs[hi[order] == 1] -= half
    vs = vals[order].astype(np.float32)
    rs = rblk[order]
    return tlo.astype(np.int64), thi.astype(np.int64), gstart, cs, vs, rs


def _pack(n_cores, nblk, prep_lp, prep_hp):
    """Per-block tile order [lp_lo | hp_lo | lp_hi | hp_hi]; build combined
    per-core idx16 / vals / rblk arrays in tile-column layout."""
    import ml_dtypes

    lp_lo, lp_hi = prep_lp[0], prep_lp[1]
    hp_lo, hp_hi = prep_hp[0], prep_hp[1]
    segs = np.stack([lp_lo, hp_lo, lp_hi, hp_hi], axis=1)  # [nblk, 4]
    tc = segs.sum(axis=1)
    tt = int(tc.sum())
    ct0 = np.concatenate([[0], np.cumsum(tc)])[:-1]
    idx_a = np.zeros((n_cores, P, tt * 8), np.int16)
    vals_a = np.zeros((n_cores, P, tt), ml_dtypes.bfloat16)
    rblk_a = np.zeros((n_cores, P, tt), ml_dtypes.bfloat16)

    preps = {0: prep_lp, 1: prep_hp}
    for c in range(n_cores):
        for b in range(nblk):
            t0 = int(ct0[b])
            # segment order: (op, half): (0,0), (1,0), (0,1), (1,1)
            seg_list = [(0, 0), (1, 0), (0, 1), (1, 1)]
            tile_off = 0
            gather_cols = {0: [], 1: []}  # half -> list of int16 idx arrays
            for si, (op, hf) in enumerate(seg_list):
                ntile = int(segs[b, si])
                if ntile == 0:
                    continue
                _, _, gstart, cs, vs, rs = preps[op]
                g = (c * nblk + b) * 2 + hf
                s, e = gstart[g], gstart[g + 1]
                npad = ntile * P
                n_real = min(int(e - s), npad)
                bc = np.zeros(npad, np.int32)
                bv = np.zeros(npad, np.float32)
                br = np.zeros(npad, np.float32)
                bc[:n_real] = cs[s : s + n_real]
                bv[:n_real] = vs[s : s + n_real]
                br[:n_real] = rs[s : s + n_real]
                tcol = t0 + tile_off
                vals_a[c, :, tcol : tcol + ntile] = (
                    bv.reshape(ntile, P).T.astype(ml_dtypes.bfloat16))
                rblk_a[c, :, tcol : tcol + ntile] = (
                    br.reshape(ntile, P).T.astype(ml_dtypes.bfloat16))
                gather_cols[hf].append(bc.astype(np.int16))
                tile_off += ntile
            # idx16 layout per gather: flat i at [i%16, i//16], replicated x8
            gpos = t0
            for hf in (0, 1):
                if not gather_cols[hf]:
                    continue
                flat = np.concatenate(gather_cols[hf])
                ntile = len(flat) // P
                arr = flat.reshape(-1, 16).T  # [16, ntile*8]
                idx_a[c, :, gpos * 8 : (gpos + ntile) * 8] = np.tile(arr, (8, 1))
                gpos += ntile
    return idx_a, vals_a, rblk_a, segs, tc, ct0, tt


# ------------------------------------------------------------- bass builder
def _build(n, d, n_cores, rpc, nblk, last_rows, segs, tc, ct0, tt, half):
    from contextlib import ExitStack

    from concourse import bacc, mybir
    from concourse import tile
    from concourse.masks import make_identity

    F32 = mybir.dt.float32
    BF16 = mybir.dt.bfloat16
    I16 = mybir.dt.int16
    Relu = mybir.ActivationFunctionType.Relu
    Sigmoid = mybir.ActivationFunctionType.Sigmoid
    Copy = mybir.ActivationFunctionType.Copy
    Alu = mybir.AluOpType

    tcmax = int(tc.max())

    nc = bacc.Bacc("TRN2", target_bir_lowering=False, debug=False,
                   num_devices=n_cores)
    x0_d = nc.dram_tensor("x0b", [half, d], BF16, kind="ExternalInput")
    x1_d = nc.dram_tensor("x1b", [n - half, d], BF16, kind="ExternalInput")
    xo_d = nc.dram_tensor("x_own", [rpc, d], F32, kind="ExternalInput")
    wt_d = nc.dram_tensor("wt", [d, d], F32, kind="ExternalInput")
    b_d = nc.dram_tensor("bvec", [1, d], F32, kind="ExternalInput")
    aw_d = nc.dram_tensor("aw", [1, d], F32, kind="ExternalInput")
    ab_d = nc.dram_tensor("ab", [1, 1], F32, kind="ExternalInput")
    idx_d = nc.dram_tensor("idx", [P, tt * 8], I16, kind="ExternalInput")
    vals_d = nc.dram_tensor("vals", [P, tt], BF16, kind="ExternalInput")
    rblk_d = nc.dram_tensor("rblk", [P, tt], BF16, kind="ExternalInput")
    out_d = nc.dram_tensor("out", [rpc, d], F32, kind="ExternalOutput")
    alpha_d = nc.dram_tensor("alpha", [rpc, 1], F32, kind="ExternalOutput")

    with tile.TileContext(nc) as tc_, ExitStack() as ctx:
        const = ctx.enter_context(tc_.tile_pool(name="const", bufs=1))
        meta = ctx.enter_context(tc_.tile_pool(name="meta", bufs=1))
        gth = ctx.enter_context(tc_.tile_pool(name="gth", bufs=3))
        pbp = ctx.enter_context(tc_.tile_pool(name="pbp", bufs=3))
        work = ctx.enter_context(tc_.tile_pool(name="work", bufs=4))
        outp = ctx.enter_context(tc_.tile_pool(name="outp", bufs=3))
        pacc = ctx.enter_context(tc_.tile_pool(name="pacc", bufs=2, space="PSUM"))
        pmisc = ctx.enter_context(tc_.tile_pool(name="pmisc", bufs=2, space="PSUM"))

        # ---- constants
        ident = const.tile([P, P], F32)
        make_identity(nc, ident[:])
        iota = const.tile([P, P * tcmax], BF16)
        nc.gpsimd.iota(iota[:], pattern=[[1, P], [0, tcmax]], base=0,
                       channel_multiplier=0,
                       allow_small_or_imprecise_dtypes=True)
        ones_col = const.tile([1, P], F32)
        nc.vector.memset(ones_col[:], 1.0)
        wt_sb = const.tile([P, d], F32)
        nc.sync.dma_start(out=wt_sb[:], in_=wt_d[:, :])
        b_sb = const.tile([1, d], F32)
        nc.sync.dma_start(out=b_sb[:], in_=b_d[:, :])
        aw_sb = const.tile([1, d], F32)
        nc.sync.dma_start(out=aw_sb[:], in_=aw_d[:, :])
        ab_sb = const.tile([1, 1], F32)
        nc.sync.dma_start(out=ab_sb[:], in_=ab_d[:, :])
        # replicate alpha_w / alpha_b across partitions via K=1 matmul
        ps_aw = pmisc.tile([P, d], F32, tag="ps_t")
        nc.tensor.matmul(ps_aw[:], lhsT=ones_col[:], rhs=aw_sb[:],
                         start=True, stop=True)
        aw_rep = const.tile([P, d], F32)
        nc.vector.tensor_copy(aw_rep[:], ps_aw[:])
        ps_ab = pmisc.tile([P, 1], F32, tag="ps_t")
        nc.tensor.matmul(ps_ab[:], lhsT=ones_col[:], rhs=ab_sb[:],
                         start=True, stop=True)
        ab_rep = const.tile([P, 1], F32)
        nc.vector.tensor_copy(ab_rep[:], ps_ab[:])

        # ---- edge metadata (whole thing resident)
        idx_sb = meta.tile([P, tt * 8], I16)
        nc.sync.dma_start(out=idx_sb[:], in_=idx_d[:, :])
        vals_sb = meta.tile([P, tt], BF16)
        nc.sync.dma_start(out=vals_sb[:], in_=vals_d[:, :])
        rblk_sb = meta.tile([P, tt], BF16)
        nc.sync.dma_start(out=rblk_sb[:], in_=rblk_d[:, :])

        alpha_all = const.tile([P, nblk], F32)
        oma_all = const.tile([P, nblk], F32)

        for b in range(nblk):
            tcb = int(tc[b])
            s0, s1, s2, s3 = (int(v) for v in segs[b])
            tl, th = s0 + s1, s2 + s3
            c0 = int(ct0[b])
            nrows = last_rows if b == nblk - 1 else P

            # ---- gather the block's x rows: low half then high half
            xg = gth.tile([P, tcmax * d], BF16, tag="xg")
            if tl > 0:
                nc.gpsimd.dma_gather(
                    out_ap=xg[:, : tl * d].rearrange("p (t f) -> p t f", f=d),
                    in_ap=x0_d[:, :],
                    idxs_ap=idx_sb[:, c0 * 8 : (c0 + tl) * 8],
                    num_idxs=tl * P, num_idxs_reg=tl * P, elem_size=d,
                    single_packet=False)
            if th > 0:
                nc.gpsimd.dma_gather(
                    out_ap=xg[:, tl * d : tcb * d].rearrange(
                        "p (t f) -> p t f", f=d),
                    in_ap=x1_d[:, :],
                    idxs_ap=idx_sb[:, (c0 + tl) * 8 : (c0 + tcb) * 8],
                    num_idxs=th * P, num_idxs_reg=th * P, elem_size=d,
                    single_packet=False)

            # ---- one-hot P'[e, (j, t)] = (j == rblk[e,t]) * val[e,t]
            pb = pbp.tile([P, P * tcmax], BF16, tag="pb")
            pb3 = pb[:].rearrange("p (j t) -> p j t", j=P, t=tcmax)[:, :, :tcb]
            iota3 = iota[:].rearrange("p (j t) -> p j t", j=P, t=tcmax)[:, :, :tcb]
            rb_b = rblk_sb[:, c0 : c0 + tcb].unsqueeze(1).to_broadcast(
                [P, P, tcb])
            va_b = vals_sb[:, c0 : c0 + tcb].unsqueeze(1).to_broadcast(
                [P, P, tcb])
            nc.vector.tensor_tensor(out=pb3, in0=iota3, in1=rb_b,
                                    op=Alu.is_equal)
            nc.vector.tensor_tensor(out=pb3, in0=pb3, in1=va_b, op=Alu.mult)

            # ---- accumulate Z_lp, Z_hp in PSUM: Z[r, f] += P'_t^T @ Xg_t
            # tile roles: [0,s0) lp | [s0,tl) hp | [tl,tl+s2) lp | rest hp
            lp_tiles = list(range(0, s0)) + list(range(tl, tl + s2))
            hp_tiles = list(range(s0, tl)) + list(range(tl + s2, tcb))
            ps_lp = pacc.tile([P, d], F32, tag="ps_lp")
            ps_hp = pacc.tile([P, d], F32, tag="ps_hp")
            for t in range(tcb):
                is_lp = t in lp_tiles
                group = lp_tiles if is_lp else hp_tiles
                ps = ps_lp if is_lp else ps_hp
                nc.tensor.matmul(
                    ps[:],
                    lhsT=pb3[:, :, t],
                    rhs=xg[:, t * d : (t + 1) * d],
                    start=(t == group[0]),
                    stop=(t == group[-1]),
                )

            # ---- alpha for this block's own rows (f32)
            xo_t = work.tile([P, d], F32, tag="xo")
            if nrows < P:
                nc.vector.memset(xo_t[:], 0.0)
            nc.sync.dma_start(out=xo_t[:nrows, :],
                              in_=xo_d[b * P : b * P + nrows, :])
            ttr = work.tile([P, d], F32, tag="ttr")
            ttr2 = work.tile([P, d], F32, tag="ttr2")
            apre = work.tile([P, 1], F32, tag="apre")
            nc.vector.tensor_tensor(out=ttr[:], in0=xo_t[:], in1=aw_rep[:],
                                    op=Alu.mult)
            nc.scalar.activation(ttr2[:], ttr[:], Copy, accum_out=apre[:])
            nc.scalar.activation(alpha_all[:, b : b + 1], apre[:],
                                 Sigmoid, bias=ab_rep[:], scale=1.0)
            nc.vector.tensor_scalar(out=oma_all[:, b : b + 1],
                                    in0=alpha_all[:, b : b + 1],
                                    scalar1=-1.0, scalar2=1.0,
                                    op0=Alu.mult, op1=Alu.add)

            # ---- mix: z = alpha * z_lp + (1 - alpha) * z_hp  (per-row scale)
            mx_lp = work.tile([P, d], F32, tag="mx_lp")
            nc.scalar.activation(mx_lp[:], ps_lp[:], Copy,
                                 scale=alpha_all[:, b : b + 1])
            mx_hp = work.tile([P, d], F32, tag="mx_hp")
            nc.scalar.activation(mx_hp[:], ps_hp[:], Copy,
                                 scale=oma_all[:, b : b + 1])
            zmix = work.tile([P, d], F32, tag="zmix")
            nc.vector.tensor_tensor(out=zmix[:], in0=mx_lp[:], in1=mx_hp[:],
                                    op=Alu.add)

            # ---- out = relu(zmix @ W^T + b): transpose zmix, then matmul
            ps_t = pmisc.tile([P, P], F32, tag="ps_t")
            nc.tensor.transpose(ps_t[:], zmix[:], ident[:])
            zt = work.tile([P, P], F32, tag="zt")
            nc.vector.tensor_copy(zt[:], ps_t[:])
            ps_o = pmisc.tile([P, d], F32, tag="ps_o")
            nc.tensor.matmul(ps_o[:], lhsT=zt[:], rhs=wt_sb[:],
                             start=True, stop=False)
            nc.tensor.matmul(ps_o[:], lhsT=ones_col[:], rhs=b_sb[:],
                             start=False, stop=True)
            o_sb = outp.tile([P, d], F32, tag="o_sb")
            nc.scalar.activation(o_sb[:], ps_o[:], Relu)
            nc.sync.dma_start(out=out_d[b * P : b * P + nrows, :],
                              in_=o_sb[:nrows, :])

        # ---- alpha output: transpose [P, nblk] -> [nblk, P] and store
        ps_at = pmisc.tile([P, P], F32, tag="ps_t")
        nc.tensor.transpose(ps_at[:nblk, :], alpha_all[:], ident[:])
        at_sb = outp.tile([P, P], F32, tag="at_sb")
        nc.vector.tensor_copy(at_sb[:nblk, :], ps_at[:nblk, :])
        nfull = nblk - 1
        if nfull > 0:
            nc.sync.dma_start(
                out=alpha_d[: nfull * P, 0].rearrange("(b r) -> b r", r=P),
                in_=at_sb[:nfull, :])
        nc.sync.dma_start(out=alpha_d[nfull * P : nfull * P + last_rows, 0]
                          .rearrange("(b r) -> b r", r=last_rows),
                          in_=at_sb[nfull : nfull + 1, :last_rows])

    nc.compile()
    return nc


# ------------------------------------------------------------------ driver
def _make(inputs, n_cores=8):
    import ml_dtypes

    x = np.asarray(inputs["x"], np.float32)
    n, d = x.shape
    half = n // 2
    rpc = n // n_cores
    nblk = -(-rpc // P)
    last_rows = rpc - (nblk - 1) * P

    prep_lp = _prep_op(inputs["lp_rows"], inputs["lp_cols"], inputs["lp_vals"],
                       n_cores, rpc, nblk, half)
    prep_hp = _prep_op(inputs["hp_rows"], inputs["hp_cols"], inputs["hp_vals"],
                       n_cores, rpc, nblk, half)
    idx_a, vals_a, rblk_a, segs, tc, ct0, tt = _pack(
        n_cores, nblk, prep_lp, prep_hp)

    nc = _build(n, d, n_cores, rpc, nblk, last_rows, segs, tc, ct0, tt, half)

    x0b = x[:half].astype(ml_dtypes.bfloat16)
    x1b = x[half:].astype(ml_dtypes.bfloat16)
    wt = np.ascontiguousarray(np.asarray(inputs["W"], np.float32).T)
    bvec = np.asarray(inputs["b"], np.float32).reshape(1, d)
    aw = np.asarray(inputs["alpha_w"], np.float32).reshape(1, d)
    ab = np.asarray(inputs["alpha_b"], np.float32).reshape(1, 1)
    in_maps = []
    for c in range(n_cores):
        in_maps.append({
            "x0b": x0b, "x1b": x1b,
            "x_own": np.ascontiguousarray(x[c * rpc : (c + 1) * rpc]),
            "wt": wt, "bvec": bvec, "aw": aw, "ab": ab,
            "idx": np.ascontiguousarray(idx_a[c]),
            "vals": np.ascontiguousarray(vals_a[c]),
            "rblk": np.ascontiguousarray(rblk_a[c]),
        })
    return nc, in_maps, rpc


def _run(inputs, n_cores=8, trace=False):
    from concourse.bass_utils import run_bass_kernel_spmd

    nc, in_maps, rpc = _make(inputs, n_cores)
    res = run_bass_kernel_spmd(nc, in_maps, core_ids=list(range(n_cores)),
                               trace=trace)
    out = np.concatenate([res.results[c]["out"] for c in range(n_cores)], 0)
    alpha = np.concatenate([res.results[c]["alpha"] for c in range(n_cores)], 0)
    return (out, alpha), res


def kernel(**inputs):
    (out, alpha), _ = _run(inputs, trace=False)
    return out, alpha


# revision 13
# speedup vs baseline: 1.0551x; 1.0551x over previous
"""AdaptiveMixGNNLayer Trainium2 kernel (8 NeuronCores, SPMD, no collectives).

Strategy: 1D node partition — each core owns a contiguous range of destination
rows (rpc = N/8). Host (inside kernel()) partitions+sorts the COO edges of both
operators by (core, 128-row destination block, x-half), packs lp and hp edges
of each (block, half) back-to-back (mixed padding — only the combined count is
padded to whole 128-edge tiles), and ships per-core int16 gather indices +
bf16 edge values. x is replicated in each core's HBM as two bf16 halves
(dma_gather indices are int16, so each gather table stays under 32768 rows).

Per destination block on device:
  - one dma_gather per half pulls x[col] for all of the block's edges
    (edge -> partition i%128, tile -> free slot i//128)
  - DVE builds one-hot P'[e, (j, t)] = val * (rblk == j) from a repeated iota
    (j-major so every operand's last AP dim is packed) via broadcast APs
  - TensorE accumulates Z[r, :] += P'_t^T @ Xg_t into PSUM. lp and hp use
    separate PSUM banks; tiles in the per-core-variable lp/hp boundary window
    are matmul'd twice with masked one-hot columns (masks are per-core data,
    so the SPMD program stays identical across cores)
  - epilogue: alpha-mix (per-row scale on ACT), TensorE transpose, @ W^T, +b,
    ReLU, DMA out.
alpha = sigmoid(x @ alpha_w^T + alpha_b) is computed on-device in f32 from the
core's own row slice.
"""
import numpy as np

P = 128  # partitions / tile edge


# ---------------------------------------------------------------- host prep
def _prep_op(rows, cols, vals, n_cores, rpc, nblk, half):
    """Sort one operator's edges by (core, dest block, col half)."""
    rows = np.asarray(rows)
    cols = np.asarray(cols)
    vals = np.asarray(vals)
    core = rows // rpc
    rloc = rows - core * rpc
    blk = rloc // P
    rblk = (rloc - blk * P).astype(np.float32)
    hi = (cols >= half).astype(np.int64)
    key = (core * nblk + blk) * 2 + hi
    order = np.argsort(key, kind="stable")
    cnt = np.bincount(key, minlength=n_cores * nblk * 2).reshape(
        n_cores, nblk, 2)
    gstart = np.concatenate([[0], np.cumsum(cnt.reshape(-1))])
    cs = cols[order].astype(np.int32)
    cs[hi[order] == 1] -= half
    vs = vals[order].astype(np.float32)
    rs = rblk[order]
    return cnt, gstart, cs, vs, rs


def _plan(n_cores, nblk, cnt_lp, cnt_hp):
    """Mixed-padding plan.

    Per (block, half): gather tiles gt = ceil(max_core(lp+hp)/128).
    The lp/hp boundary inside those tiles varies per core, so tiles in
    [min_core(lp)//128, ceil(max_core(lp)/128)) are "mixed" and get two
    matmuls (an lp meta-column and an hp meta-column, masked per core via
    val=0); tiles below are pure lp, above pure hp.

    Returns per (block, half): gt, lo_t (first possibly-mixed tile),
    hi_t (first pure-hp tile), plus meta-column counts.
    """
    tot = cnt_lp + cnt_hp  # [ncores, nblk, 2]
    gt = -(-tot.max(axis=0) // P)  # [nblk, 2]
    gt[gt.sum(axis=1) == 0, 0] = 1  # every block needs >= 1 tile
    lp_min = cnt_lp.min(axis=0)  # [nblk, 2]
    lp_max = cnt_lp.max(axis=0)
    lo_t = lp_min // P
    hi_t = np.minimum(-(-lp_max // P), gt)
    lo_t = np.minimum(lo_t, gt)
    # every block needs >= 1 lp-capable and >= 1 hp-capable meta column so
    # both PSUM groups have at least one matmul
    # meta cols per (blk, half): pure-lp: lo_t, mixed: (hi_t-lo_t)*2, rest
    return gt, lo_t, hi_t


def _pack(n_cores, nblk, prep_lp, prep_hp):
    import ml_dtypes

    cnt_lp, gs_lp, cs_lp, vs_lp, rs_lp = prep_lp
    cnt_hp, gs_hp, cs_hp, vs_hp, rs_hp = prep_hp
    gt, lo_t, hi_t = _plan(n_cores, nblk, cnt_lp, cnt_hp)

    # per (block, half) meta-column layout: for tile t:
    #   t < lo_t: 1 col (lp) | lo_t <= t < hi_t: 2 cols (lp, hp) | else 1 (hp)
    # block structure tables (shared across cores):
    plans = []  # per block: list of (half, tile, mcol_lp or None, mcol_hp or None)
    mc_total = 0
    gt_tot = gt.sum(axis=1)  # gather tiles per block
    gct0 = np.concatenate([[0], np.cumsum(gt_tot)])[:-1]  # gather col base
    for b in range(nblk):
        ents = []
        for hf in (0, 1):
            for t in range(int(gt[b, hf])):
                has_lp = t < hi_t[b, hf]
                has_hp = t >= lo_t[b, hf]
                mlp = mc_total if has_lp else None
                if has_lp:
                    mc_total += 1
                mhp = mc_total if has_hp else None
                if has_hp:
                    mc_total += 1
                ents.append((hf, t, mlp, mhp))
        # guarantee each operator has at least one matmul in this block
        if not any(e[2] is not None for e in ents):
            hf, t, _, mhp = ents[0]
            ents[0] = (hf, t, mc_total, mhp)
            mc_total += 1
        if not any(e[3] is not None for e in ents):
            hf, t, mlp, _ = ents[-1]
            ents[-1] = (hf, t, mlp, mc_total)
            mc_total += 1
        plans.append(ents)

    tt = int(gt_tot.sum())  # gather tile columns
    idx_a = np.zeros((n_cores, P, tt * 8), np.int16)
    vals_a = np.zeros((n_cores, P, mc_total), ml_dtypes.bfloat16)
    rblk_a = np.zeros((n_cores, P, mc_total), ml_dtypes.bfloat16)

    for c in range(n_cores):
        for b in range(nblk):
            for hf in (0, 1):
                ntile = int(gt[b, hf])
                if ntile == 0:
                    continue
                npad = ntile * P
                g = (c * nblk + b) * 2 + hf
                sl, el = gs_lp[g], gs_lp[g + 1]
                sh, eh = gs_hp[g], gs_hp[g + 1]
                nl, nh = int(el - sl), int(eh - sh)
                bc = np.zeros(npad, np.int32)
                bv = np.zeros(npad, np.float32)
                br = np.zeros(npad, np.float32)
                is_lp = np.zeros(npad, bool)
                bc[:nl] = cs_lp[sl:el]
                bv[:nl] = vs_lp[sl:el]
                br[:nl] = rs_lp[sl:el]
                is_lp[:nl] = True
                bc[nl : nl + nh] = cs_hp[sh:eh]
                bv[nl : nl + nh] = vs_hp[sh:eh]
                br[nl : nl + nh] = rs_hp[sh:eh]
                # gather idx cols
                g0 = int(gct0[b] + (gt[b, 0] if hf else 0))
                arr = bc.astype(np.int16).reshape(-1, 16).T
                idx_a[c, :, g0 * 8 : (g0 + ntile) * 8] = np.tile(arr, (8, 1))
                # meta cols from the block plan
                bv2 = bv.reshape(ntile, P)
                br2 = br.reshape(ntile, P)
                il2 = is_lp.reshape(ntile, P)
                for hf2, t, mlp, mhp in plans[b]:
                    if hf2 != hf:
                        continue
                    if mlp is not None:
                        v = np.where(il2[t], bv2[t], 0.0)
                        vals_a[c, :, mlp] = v.astype(ml_dtypes.bfloat16)
                        rblk_a[c, :, mlp] = br2[t].astype(ml_dtypes.bfloat16)
                    if mhp is not None:
                        v = np.where(il2[t], 0.0, bv2[t])
                        vals_a[c, :, mhp] = v.astype(ml_dtypes.bfloat16)
                        rblk_a[c, :, mhp] = br2[t].astype(ml_dtypes.bfloat16)
    return idx_a, vals_a, rblk_a, gt, gct0, plans, tt, mc_total


# ------------------------------------------------------------- bass builder
def _build(n, d, n_cores, rpc, nblk, last_rows, gt, gct0, plans, tt, mc_total,
           half):
    from contextlib import ExitStack

    from concourse import bacc, mybir
    from concourse import tile
    from concourse.masks import make_identity

    F32 = mybir.dt.float32
    BF16 = mybir.dt.bfloat16
    I16 = mybir.dt.int16
    Relu = mybir.ActivationFunctionType.Relu
    Sigmoid = mybir.ActivationFunctionType.Sigmoid
    Copy = mybir.ActivationFunctionType.Copy
    Alu = mybir.AluOpType

    # meta columns per block (for the one-hot build batching)
    mct = [len([1 for e in p for m in (e[2], e[3]) if m is not None])
           for p in plans]
    mc0 = np.concatenate([[0], np.cumsum(mct)])[:-1]
    mcmax = max(mct)
    gtmax = int(gt.sum(axis=1).max())

    nc = bacc.Bacc("TRN2", target_bir_lowering=False, debug=False,
                   num_devices=n_cores)
    x0_d = nc.dram_tensor("x0b", [half, d], BF16, kind="ExternalInput")
    x1_d = nc.dram_tensor("x1b", [n - half, d], BF16, kind="ExternalInput")
    xo_d = nc.dram_tensor("x_own", [rpc, d], F32, kind="ExternalInput")
    wt_d = nc.dram_tensor("wt", [d, d], F32, kind="ExternalInput")
    b_d = nc.dram_tensor("bvec", [1, d], F32, kind="ExternalInput")
    aw_d = nc.dram_tensor("aw", [1, d], F32, kind="ExternalInput")
    ab_d = nc.dram_tensor("ab", [1, 1], F32, kind="ExternalInput")
    idx_d = nc.dram_tensor("idx", [P, tt * 8], I16, kind="ExternalInput")
    vals_d = nc.dram_tensor("vals", [P, mc_total], BF16, kind="ExternalInput")
    rblk_d = nc.dram_tensor("rblk", [P, mc_total], BF16, kind="ExternalInput")
    out_d = nc.dram_tensor("out", [rpc, d], F32, kind="ExternalOutput")
    alpha_d = nc.dram_tensor("alpha", [rpc, 1], F32, kind="ExternalOutput")

    with tile.TileContext(nc) as tc_, ExitStack() as ctx:
        const = ctx.enter_context(tc_.tile_pool(name="const", bufs=1))
        meta = ctx.enter_context(tc_.tile_pool(name="meta", bufs=1))
        gth = ctx.enter_context(tc_.tile_pool(name="gth", bufs=3))
        pbp = ctx.enter_context(tc_.tile_pool(name="pbp", bufs=3))
        work = ctx.enter_context(tc_.tile_pool(name="work", bufs=4))
        outp = ctx.enter_context(tc_.tile_pool(name="outp", bufs=3))
        pacc = ctx.enter_context(tc_.tile_pool(name="pacc", bufs=2, space="PSUM"))
        pmisc = ctx.enter_context(tc_.tile_pool(name="pmisc", bufs=2, space="PSUM"))

        # ---- constants
        ident = const.tile([P, P], F32)
        make_identity(nc, ident[:])
        iota = const.tile([P, P * mcmax], BF16)
        nc.gpsimd.iota(iota[:], pattern=[[1, P], [0, mcmax]], base=0,
                       channel_multiplier=0,
                       allow_small_or_imprecise_dtypes=True)
        ones_col = const.tile([1, P], F32)
        nc.vector.memset(ones_col[:], 1.0)
        wt_sb = const.tile([P, d], F32)
        nc.sync.dma_start(out=wt_sb[:], in_=wt_d[:, :])
        b_sb = const.tile([1, d], F32)
        nc.sync.dma_start(out=b_sb[:], in_=b_d[:, :])
        aw_sb = const.tile([1, d], F32)
        nc.sync.dma_start(out=aw_sb[:], in_=aw_d[:, :])
        ab_sb = const.tile([1, 1], F32)
        nc.sync.dma_start(out=ab_sb[:], in_=ab_d[:, :])
        ps_aw = pmisc.tile([P, d], F32, tag="ps_t")
        nc.tensor.matmul(ps_aw[:], lhsT=ones_col[:], rhs=aw_sb[:],
                         start=True, stop=True)
        aw_rep = const.tile([P, d], F32)
        nc.vector.tensor_copy(aw_rep[:], ps_aw[:])
        ps_ab = pmisc.tile([P, 1], F32, tag="ps_t")
        nc.tensor.matmul(ps_ab[:], lhsT=ones_col[:], rhs=ab_sb[:],
                         start=True, stop=True)
        ab_rep = const.tile([P, 1], F32)
        nc.vector.tensor_copy(ab_rep[:], ps_ab[:])

        # ---- edge metadata (fully resident)
        idx_sb = meta.tile([P, tt * 8], I16)
        nc.sync.dma_start(out=idx_sb[:], in_=idx_d[:, :])
        vals_sb = meta.tile([P, mc_total], BF16)
        nc.sync.dma_start(out=vals_sb[:], in_=vals_d[:, :])
        rblk_sb = meta.tile([P, mc_total], BF16)
        nc.sync.dma_start(out=rblk_sb[:], in_=rblk_d[:, :])

        alpha_all = const.tile([P, nblk], F32)
        oma_all = const.tile([P, nblk], F32)

        for b in range(nblk):
            gl, gh = int(gt[b, 0]), int(gt[b, 1])
            gtb = gl + gh
            g0 = int(gct0[b])
            m0 = int(mc0[b])
            mcb = mct[b]
            nrows = last_rows if b == nblk - 1 else P

            # ---- gather the block's x rows: low half then high half
            xg = gth.tile([P, gtmax * d], BF16, tag="xg")
            if gl > 0:
                nc.gpsimd.dma_gather(
                    out_ap=xg[:, : gl * d].rearrange("p (t f) -> p t f", f=d),
                    in_ap=x0_d[:, :],
                    idxs_ap=idx_sb[:, g0 * 8 : (g0 + gl) * 8],
                    num_idxs=gl * P, num_idxs_reg=gl * P, elem_size=d,
                    single_packet=False)
            if gh > 0:
                nc.gpsimd.dma_gather(
                    out_ap=xg[:, gl * d : gtb * d].rearrange(
                        "p (t f) -> p t f", f=d),
                    in_ap=x1_d[:, :],
                    idxs_ap=idx_sb[:, (g0 + gl) * 8 : (g0 + gtb) * 8],
                    num_idxs=gh * P, num_idxs_reg=gh * P, elem_size=d,
                    single_packet=False)

            # ---- one-hot P'[e, (j, m)] = (j == rblk[e,m]) * val[e,m]
            pb = pbp.tile([P, P * mcmax], BF16, tag="pb")
            pb3 = pb[:].rearrange("p (j t) -> p j t", j=P, t=mcmax)[:, :, :mcb]
            iota3 = iota[:].rearrange("p (j t) -> p j t", j=P, t=mcmax)[:, :, :mcb]
            rb_b = rblk_sb[:, m0 : m0 + mcb].unsqueeze(1).to_broadcast(
                [P, P, mcb])
            va_b = vals_sb[:, m0 : m0 + mcb].unsqueeze(1).to_broadcast(
                [P, P, mcb])
            nc.vector.tensor_tensor(out=pb3, in0=iota3, in1=rb_b,
                                    op=Alu.is_equal)
            nc.vector.tensor_tensor(out=pb3, in0=pb3, in1=va_b, op=Alu.mult)

            # ---- accumulate Z_lp, Z_hp in PSUM: Z[r, :] += P'_m^T @ Xg_t
            ps_lp = pacc.tile([P, d], F32, tag="ps_lp")
            ps_hp = pacc.tile([P, d], F32, tag="ps_hp")
            # emission order: plan entries in order; lp/hp groups tracked
            lp_ms = [e[2] for e in plans[b] if e[2] is not None]
            hp_ms = [e[3] for e in plans[b] if e[3] is not None]
            for hf2, t, mlp, mhp in plans[b]:
                gtile = t + (gl if hf2 else 0)
                rhs = xg[:, gtile * d : (gtile + 1) * d]
                for mcol, ps, grp in ((mlp, ps_lp, lp_ms), (mhp, ps_hp, hp_ms)):
                    if mcol is None:
                        continue
                    nc.tensor.matmul(
                        ps[:],
                        lhsT=pb3[:, :, mcol - m0],
                        rhs=rhs,
                        start=(mcol == grp[0]),
                        stop=(mcol == grp[-1]),
                    )

            # ---- alpha for this block's own rows (f32)
            xo_t = work.tile([P, d], F32, tag="xo")
            if nrows < P:
                nc.vector.memset(xo_t[:], 0.0)
            nc.sync.dma_start(out=xo_t[:nrows, :],
                              in_=xo_d[b * P : b * P + nrows, :])
            ttr = work.tile([P, d], F32, tag="ttr")
            ttr2 = work.tile([P, d], F32, tag="ttr2")
            apre = work.tile([P, 1], F32, tag="apre")
            nc.vector.tensor_tensor(out=ttr[:], in0=xo_t[:], in1=aw_rep[:],
                                    op=Alu.mult)
            nc.scalar.activation(ttr2[:], ttr[:], Copy, accum_out=apre[:])
            nc.scalar.activation(alpha_all[:, b : b + 1], apre[:],
                                 Sigmoid, bias=ab_rep[:], scale=1.0)
            nc.vector.tensor_scalar(out=oma_all[:, b : b + 1],
                                    in0=alpha_all[:, b : b + 1],
                                    scalar1=-1.0, scalar2=1.0,
                                    op0=Alu.mult, op1=Alu.add)

            # ---- mix: z = alpha * z_lp + (1 - alpha) * z_hp
            mx_lp = work.tile([P, d], F32, tag="mx_lp")
            nc.scalar.activation(mx_lp[:], ps_lp[:], Copy,
                                 scale=alpha_all[:, b : b + 1])
            mx_hp = work.tile([P, d], F32, tag="mx_hp")
            nc.scalar.activation(mx_hp[:], ps_hp[:], Copy,
                                 scale=oma_all[:, b : b + 1])
            zmix = work.tile([P, d], F32, tag="zmix")
            nc.vector.tensor_tensor(out=zmix[:], in0=mx_lp[:], in1=mx_hp[:],
                                    op=Alu.add)

            # ---- out = relu(zmix @ W^T + b)
            ps_t = pmisc.tile([P, P], F32, tag="ps_t")
            nc.tensor.transpose(ps_t[:], zmix[:], ident[:])
            zt = work.tile([P, P], F32, tag="zt")
            nc.vector.tensor_copy(zt[:], ps_t[:])
            ps_o = pmisc.tile([P, d], F32, tag="ps_o")
            nc.tensor.matmul(ps_o[:], lhsT=zt[:], rhs=wt_sb[:],
                             start=True, stop=False)
            nc.tensor.matmul(ps_o[:], lhsT=ones_col[:], rhs=b_sb[:],
                             start=False, stop=True)
            o_sb = outp.tile([P, d], F32, tag="o_sb")
            nc.scalar.activation(o_sb[:], ps_o[:], Relu)
            nc.sync.dma_start(out=out_d[b * P : b * P + nrows, :],
                              in_=o_sb[:nrows, :])

        # ---- alpha output: transpose [P, nblk] -> [nblk, P] and store
        ps_at = pmisc.tile([P, P], F32, tag="ps_t")
        nc.tensor.transpose(ps_at[:nblk, :], alpha_all[:], ident[:])
        at_sb = outp.tile([P, P], F32, tag="at_sb")
        nc.vector.tensor_copy(at_sb[:nblk, :], ps_at[:nblk, :])
        nfull = nblk - 1
        if nfull > 0:
            nc.sync.dma_start(
                out=alpha_d[: nfull * P, 0].rearrange("(b r) -> b r", r=P),
                in_=at_sb[:nfull, :])
        nc.sync.dma_start(out=alpha_d[nfull * P : nfull * P + last_rows, 0]
                          .rearrange("(b r) -> b r", r=last_rows),
                          in_=at_sb[nfull : nfull + 1, :last_rows])

    nc.compile()
    return nc


# ------------------------------------------------------------------ driver
def _make(inputs, n_cores=8):
    import ml_dtypes

    x = np.asarray(inputs["x"], np.float32)
    n, d = x.shape
    half = n // 2
    rpc = n // n_cores
    nblk = -(-rpc // P)
    last_rows = rpc - (nblk - 1) * P

    prep_lp = _prep_op(inputs["lp_rows"], inputs["lp_cols"], inputs["lp_vals"],
                       n_cores, rpc, nblk, half)
    prep_hp = _prep_op(inputs["hp_rows"], inputs["hp_cols"], inputs["hp_vals"],
                       n_cores, rpc, nblk, half)
    idx_a, vals_a, rblk_a, gt, gct0, plans, tt, mc_total = _pack(
        n_cores, nblk, prep_lp, prep_hp)

    nc = _build(n, d, n_cores, rpc, nblk, last_rows, gt, gct0, plans, tt,
                mc_total, half)

    x0b = x[:half].astype(ml_dtypes.bfloat16)
    x1b = x[half:].astype(ml_dtypes.bfloat16)
    wt = np.ascontiguousarray(np.asarray(inputs["W"], np.float32).T)
    bvec = np.asarray(inputs["b"], np.float32).reshape(1, d)
    aw = np.asarray(inputs["alpha_w"], np.float32).reshape(1, d)
    ab = np.asarray(inputs["alpha_b"], np.float32).reshape(1, 1)
    in_maps = []
    for c in range(n_cores):
        in_maps.append({
            "x0b": x0b, "x1b": x1b,
            "x_own": np.ascontiguousarray(x[c * rpc : (c + 1) * rpc]),
            "wt": wt, "bvec": bvec, "aw": aw, "ab": ab,
            "idx": np.ascontiguousarray(idx_a[c]),
            "vals": np.ascontiguousarray(vals_a[c]),
            "rblk": np.ascontiguousarray(rblk_a[c]),
        })
    return nc, in_maps, rpc


def _run(inputs, n_cores=8, trace=False):
    from concourse.bass_utils import run_bass_kernel_spmd

    nc, in_maps, rpc = _make(inputs, n_cores)
    res = run_bass_kernel_spmd(nc, in_maps, core_ids=list(range(n_cores)),
                               trace=trace)
    out = np.concatenate([res.results[c]["out"] for c in range(n_cores)], 0)
    alpha = np.concatenate([res.results[c]["alpha"] for c in range(n_cores)], 0)
    return (out, alpha), res


def kernel(**inputs):
    (out, alpha), _ = _run(inputs, trace=False)
    return out, alpha
